# revision 57
# baseline (speedup 1.0000x reference)
"""GMM log-likelihood kernel for Trainium2 (Bass/Tile), 8-core data-parallel.

Math (host precompute in f64):
  B_k = L_k^{-1} (Cholesky inverse),  c_k = B_k mu_k
  wlp_k(x) = -0.5*||B_k x - c_k||^2 + K_k,
      K_k = log w_k - d/2 log 2pi - half_logdet_k
  lse(x)   = m0 + log(sum_k exp(wlp_k - m0))
  out      = sum_x lse(x)

Device pipeline per 256-sample pair (98 pairs/core, samples padded with 0):
  DMA:  one host-pretransposed [65, 256] pair block (64 feature rows +
        ones row carrying the -c shift); consts issued from the ACT queue
  PE:   y' = [B|-c]^T xt  ->  one PSUM pair-tile [128, 2048] (4x 512-col MMs)
  ACT:  one Derivative_Erf pass = (2/sqrt(pi)) exp(-y'^2/2) Gaussian
        factors, PSUM -> SBUF bf16 (fuses square AND exp in one pass)
  DVE:  two pairwise-halving TT products (bf16 2x mode) 64->16 wide, then
        a 1x grouped product-reduce to [128, 32] f32
  GPSIMD: multiplies in exp(K_k - m0) (sqrt(pi)/2)^64 off the critical path
Phase 2: wbuf already holds exp(wlp - m0); halve + component sums (two
  overlapped halves), one Ln with accum_out, ones-matmul to a scalar.
  Host adds m0*count and subtracts the 88 zero-pad samples' closed-form lse.
"""

import numpy as np

N_COMPONENTS = 16
N_FEATURES = 64
N_SAMPLES = 200000
N_CORES = 8
PER_CORE = N_SAMPLES // N_CORES          # 25000
TILE_P = 128
N_TILES = -(-PER_CORE // TILE_P)         # 196 (ceil)
N_PAIRS = (N_TILES + 1) // 2             # 98
KD = N_COMPONENTS * N_FEATURES           # 1024
GRP = N_FEATURES + 1                     # 65 (64 features + ones row)

_CACHE = {}


def _build_nc(n_pairs):
    import concourse.tile as tile
    from concourse import bacc, mybir

    n_tiles = n_pairs * 2
    f32 = mybir.dt.float32
    bf16 = mybir.dt.bfloat16

    nc = bacc.Bacc("TRN2", target_bir_lowering=False, debug=False,
                   num_devices=N_CORES)

    xpt = nc.dram_tensor("xpt", [n_pairs * GRP, 2 * TILE_P], bf16,
                         kind="ExternalInput").ap()
    bm65 = nc.dram_tensor("bm65", [GRP, KD], bf16, kind="ExternalInput").ap()
    vst = nc.dram_tensor("vst", [128, 4 * N_COMPONENTS], f32,
                         kind="ExternalInput").ap()
    ones = nc.dram_tensor("ones", [128, 1], f32, kind="ExternalInput").ap()
    out = nc.dram_tensor("out", [1, 1], f32, kind="ExternalOutput").ap()

    W = n_tiles * N_COMPONENTS
    G2 = 2 * N_COMPONENTS                # 32 groups per pair

    with tile.TileContext(nc) as tc:
        with (
            tc.tile_pool(name="const", bufs=1) as const_pool,
            tc.tile_pool(name="work", bufs=4) as work_pool,
            tc.tile_pool(name="yp", bufs=2, space="PSUM") as yp_pool,
        ):
            wbuf_pool = xin_pool = ysq_pool = tb_pool = work_pool
            # consts issued from compute engines so the Sync queue reaches
            # the first data tile's DMA immediately after its prologue
            bm = const_pool.tile([GRP, KD], bf16)
            nc.scalar.dma_start(bm[:], bm65[:])
            vsts = const_pool.tile([128, 2 * G2], f32)
            nc.scalar.dma_start(vsts[:], vst[:])
            on1 = const_pool.tile([128, 1], f32)
            nc.scalar.dma_start(on1[:], ones[:])

            wbuf = wbuf_pool.tile([128, W], bf16, bufs=1)
            ebs = wbuf_pool.tile([128, W // 2], bf16, bufs=1)
            rsum = const_pool.tile([128, n_tiles], f32)
            lnr = const_pool.tile([128, n_tiles], f32)
            csum = const_pool.tile([128, 1], f32)

            def phase2(tlo, thi):
                """lse for tiles [tlo, thi): wbuf already holds
                exp(wlp - m0); halve, comp-sum, ln+accum."""
                h8 = N_COMPONENTS // 2
                tmid = (tlo + thi) // 2
                for lo, hi in ((tlo, tmid), (tmid, thi)):
                    ev = wbuf[:, lo * N_COMPONENTS:hi * N_COMPONENTS].rearrange(
                        "p (t k) -> p t k", k=N_COMPONENTS)
                    esv = ebs[:, lo * h8:hi * h8].rearrange(
                        "p (t k) -> p t k", k=h8)
                    nc.vector.tensor_add(esv, ev[:, :, 0:h8],
                                         ev[:, :, h8:N_COMPONENTS])
                    nc.vector.reduce_sum(rsum[:, lo:hi], esv,
                                         axis=mybir.AxisListType.X)
                nc.scalar.activation(lnr[:, tlo:thi], rsum[:, tlo:thi],
                                     mybir.ActivationFunctionType.Ln,
                                     accum_out=csum[:])

            third = n_pairs // 3
            for p in range(n_pairs):
                xt2 = xin_pool.tile([GRP, 2 * TILE_P], bf16, tag="xt2")
                nc.sync.dma_start(xt2[:], xpt[p * GRP:(p + 1) * GRP, :])
                yp = yp_pool.tile([128, 2 * KD], f32, tag="yp")
                for h in range(2):
                    o = h * KD
                    lhs = xt2[:, h * TILE_P:(h + 1) * TILE_P]
                    nc.tensor.matmul(yp[:, o:o + 512], lhs, bm[:, 0:512])
                    nc.tensor.matmul(yp[:, o + 512:o + 1024], lhs,
                                     bm[:, 512:1024])

                # Gaussian factors (2/sqrt(pi)) * exp(-0.5 y^2) in one ACT
                # pass -- no separate exp stage needed later
                ysq = ysq_pool.tile([128, 2 * KD], bf16, tag="ysq")
                nc.scalar.activation(ysq[:], yp[:],
                                     mybir.ActivationFunctionType.Derivative_Erf,
                                     scale=float(np.sqrt(0.5)))

                # 64 -> 32 -> 16 wide pairwise products (bf16, 2x DVE mode)
                tb = tb_pool.tile([128, 1536], bf16, tag="tb")
                if p % 2 == 0:
                    sred2 = tb_pool.tile([128, 2 * G2], f32, tag="sred")
                s1 = tb[:, 0:1024].rearrange("p (g e) -> p g e", e=32)
                ys = ysq[:].rearrange("p (g e) -> p g e", e=64)
                nc.vector.tensor_mul(s1, ys[:, :, 0:32], ys[:, :, 32:64])
                s2 = tb[:, 1024:1536].rearrange("p (g e) -> p g e", e=16)
                nc.vector.tensor_mul(s2, s1[:, :, 0:16], s1[:, :, 16:32])
                half = (p % 2) * G2
                nc.vector.tensor_reduce(sred2[:, half:half + G2], s2,
                                        axis=mybir.AxisListType.X,
                                        op=mybir.AluOpType.mult)
                if p % 2 == 1:
                    # off the critical path (wbuf is read only in phase 2):
                    # run on the otherwise-idle GPSIMD to keep the in-order
                    # DVE queue at its short per-pair sequence
                    col = 2 * (p - 1) * N_COMPONENTS
                    nc.gpsimd.tensor_mul(wbuf[:, col:col + 2 * G2],
                                         sred2[:], vsts[:])

            phase2(0, n_tiles)  # single pass: one Exp/Ln table round-trip

            rp = yp_pool.tile([1, 1], f32, tag="yp")
            nc.tensor.matmul(rp[:], on1[:], csum[:])
            res = const_pool.tile([1, 1], f32)
            nc.scalar.copy(res[:], rp[:])
            nc.sync.dma_start(out[:], res[:])

    nc.compile()
    return nc


def _precompute(weights, means, covariances):
    """Host-side O(K d^3) prep in float64. Returns (bm65, vst_row, m0)."""
    import ml_dtypes

    K, d = means.shape
    L = np.linalg.cholesky(covariances.astype(np.float64))
    half_logdet = np.log(np.diagonal(L, axis1=-2, axis2=-1)).sum(-1)
    eye = np.eye(d)
    B = np.stack([np.linalg.solve(L[k], eye) for k in range(K)])  # L^-1
    mu = means.astype(np.float64)
    c = np.einsum('kij,kj->ki', B, mu)                            # B mu
    Kconst = (np.log(weights.astype(np.float64))
              - 0.5 * d * np.log(2.0 * np.pi) - half_logdet)
    m0 = float(Kconst.max()) - 20.0
    # per-component multiplier: exp(K - m0) * (sqrt(pi)/2)^d undoes the
    # (2/sqrt(pi))^d prefactor of the Derivative_Erf Gaussian products
    v = np.exp(Kconst - m0 + d * np.log(np.sqrt(np.pi) / 2.0))    # [K]

    bm65 = np.zeros((GRP, KD), np.float32)
    for k in range(K):
        bm65[0:d, k * d:(k + 1) * d] = B[k].T.astype(np.float32)
        bm65[d, k * d:(k + 1) * d] = -c[k].astype(np.float32)
    bm65 = bm65.astype(ml_dtypes.bfloat16)
    vst = np.tile(v.astype(np.float32), 4)                        # [64]
    vst_row = np.broadcast_to(vst, (128, 4 * N_COMPONENTS)).copy()

    # lse of a zero-padded sample (device pads with x=0, ones row = 1):
    # y' = -c exactly as the device sees it (bf16 weights), rest in f64
    cb = -bm65[d].astype(np.float64).reshape(K, d)                # bf16(-c)
    lse0 = m0 + float(np.log(np.exp(Kconst - m0
                                    - 0.5 * (cb * cb).sum(-1)).sum()))
    return bm65, vst_row, m0, lse0


def _make_inputs(data, bm65, vst_row, n_tiles):
    """Build the 8 per-core input maps (host-pretransposed tiles)."""
    import ml_dtypes

    ones = np.ones((128, 1), np.float32)

    padded = n_tiles * TILE_P
    in_maps = []
    for cidx in range(N_CORES):
        sl = np.asarray(data[cidx * PER_CORE:(cidx + 1) * PER_CORE],
                        np.float32)
        xp = np.zeros((padded, N_FEATURES), np.float32)
        xp[:sl.shape[0]] = sl
        # pairs of tiles side by side: [n_pairs, 65, 256]
        # (64 feature rows + ones row; cols = 2x128 samples)
        n_pairs = n_tiles // 2
        xt = xp.reshape(n_pairs, 2, TILE_P, N_FEATURES).transpose(0, 3, 1, 2)
        xpt = np.ones((n_pairs, GRP, 2 * TILE_P), np.float32)
        xpt[:, :N_FEATURES, :] = xt.reshape(n_pairs, N_FEATURES, 2 * TILE_P)
        xpt = xpt.reshape(n_pairs * GRP, 2 * TILE_P).astype(ml_dtypes.bfloat16)
        in_maps.append({"xpt": xpt, "bm65": bm65, "vst": vst_row,
                        "ones": ones})
    return in_maps


def _run(data, weights, means, covariances, trace=False):
    from concourse.bass_utils import run_bass_kernel_spmd

    bm65, vst_row, m0, lse0 = _precompute(np.asarray(weights),
                                          np.asarray(means),
                                          np.asarray(covariances))
    if "nc" not in _CACHE:
        _CACHE["nc"] = _build_nc(N_PAIRS)
    nc = _CACHE["nc"]

    in_maps = _make_inputs(data, bm65, vst_row, N_TILES)
    res = run_bass_kernel_spmd(nc, in_maps, list(range(N_CORES)), trace=trace)
    n_pad = N_TILES * TILE_P - PER_CORE                           # 88
    total = 0.0
    for cidx in range(N_CORES):
        total += (float(res.results[cidx]["out"][0, 0])
                  + PER_CORE * m0 - n_pad * (lse0 - m0))
    return np.float32(total), res


def kernel(data, weights, means, covariances):
    return _run(data, weights, means, covariances)[0]
